# revision 1
# baseline (speedup 1.0000x reference)
"""Bass kernel builder for nn_Attention (channel attention / XCA block).

Per-core computation (one batch element, data-parallel over batch=8):
  qkv1 = w_qkv @ x            (1x1 conv, 576x192 @ 192x16384)
  qkv  = depthwise3x3(qkv1)   (per-channel 3x3, SAME zero pad)
  q,k,v = split(qkv)
  q,k l2-normalized per channel; G = q @ k^T per head (48x48 over n=16384)
  attn = softmax(G * temp); out = w_proj @ blockdiag(attn) @ v

Implementation notes:
- All heavy math in bf16 on TensorE, f32 PSUM accumulation.
- The depthwise conv runs on TensorE as 9 shifted matmuls with diagonal
  weight matrices (host-built), accumulating in PSUM. Spatial x-wraps at
  row edges are corrected by small DVE scalar_tensor_tensor fixups.
- q/k tiles are PE-transposed per image row to feed the per-head gram
  accumulation (K = spatial on partitions).
- attn @ v is folded with the projection: out = (w_proj @ blockdiag(attn)) @ v.
"""

import sys

sys.path.insert(0, "/opt/trn_rl_repo")

import contextlib

import numpy as np
import ml_dtypes

import concourse.bass as bass
import concourse.tile as tile
from concourse import mybir
from concourse.tile import add_dep_helper

BF16 = mybir.dt.bfloat16
F32 = mybir.dt.float32

C = 192           # channels
OC = 3 * C        # qkv channels = 576
HEADS = 4
HC = C // HEADS   # 48
HW = 128          # image height/width
N = HW * HW       # 16384 spatial
H_STRIP = 16      # rows per strip
NSTRIP = HW // H_STRIP
OUTC = H_STRIP * 128          # dw output columns per strip
PAD = (H_STRIP + 2) * 128 + 4  # padded strip slab; data at [2, 2+(H+2)*128)

# channel chunking of the 576 qkv channels: 4 x 128 + 64
CHUNKS = [(0, 128), (128, 128), (256, 128), (384, 128), (512, 64)]
# dw taps: (dy, dx), slab shift = dy*128 + dx
TAPS = [(dy, dx) for dy in (-1, 0, 1) for dx in (-1, 0, 1)]


def prep_weights(w_qkv, w_dw, temperature, w_proj):
    """Host-side weight layout prep. Returns dict of numpy arrays."""
    w_qkv = np.asarray(w_qkv, np.float32)
    w_dw = np.asarray(w_dw, np.float32).reshape(OC, 3, 3)
    w_proj = np.asarray(w_proj, np.float32)
    temperature = np.asarray(temperature, np.float32).reshape(HEADS)

    out = {}
    w1T = np.ascontiguousarray(w_qkv.T)  # (192, 576)
    out["w1Ta"] = w1T[:128].astype(ml_dtypes.bfloat16)
    out["w1Tb"] = np.ascontiguousarray(w1T[128:]).astype(ml_dtypes.bfloat16)

    # diag matrices for dw taps, per 128-chunk. col index m*128 + c, m = oc*9+tap
    rd = np.zeros((128, 27 * 128), np.float32)
    for oc in range(3):
        base, sz = CHUNKS[oc]
        for t, (dy, dx) in enumerate(TAPS):
            m = oc * 9 + t
            np.fill_diagonal(rd[:, m * 128:(m + 1) * 128],
                             w_dw[base:base + sz, 1 + dy, 1 + dx])
    out["rdiag"] = rd.astype(ml_dtypes.bfloat16)

    rdb = np.zeros((64, 9 * 64), np.float32)
    base, sz = CHUNKS[4]
    for t, (dy, dx) in enumerate(TAPS):
        np.fill_diagonal(rdb[:, t * 64:(t + 1) * 64],
                         w_dw[base:base + sz, 1 + dy, 1 + dx])
    out["rdiagb"] = rdb.astype(ml_dtypes.bfloat16)


    # negated tap weights for edge fixups: (128, 45) f32; col = oc*9 + tap.
    # IMPORTANT: must negate the bf16-rounded weight so the fixup subtracts
    # exactly what the bf16 diag matmul added.
    wneg = np.zeros((128, 45), np.float32)
    for oc in range(5):
        base, sz = CHUNKS[oc]
        for t, (dy, dx) in enumerate(TAPS):
            wb = w_dw[base:base + sz, 1 + dy, 1 + dx].astype(ml_dtypes.bfloat16)
            wneg[:sz, oc * 9 + t] = -wb.astype(np.float32)
    out["wneg"] = wneg
    out["wtaps"] = -wneg

    # w_proj^T per head: (48, 4*192); [p, h*192+o] = w_proj[o, h*48+p]
    wpTh = np.zeros((HC, HEADS * C), np.float32)
    for h in range(HEADS):
        wpTh[:, h * C:(h + 1) * C] = w_proj[:, h * HC:(h + 1) * HC].T
    out["wpTh"] = wpTh.astype(ml_dtypes.bfloat16)

    # temperatures broadcast per partition: (48, 4)
    out["temps"] = np.ascontiguousarray(
        np.broadcast_to(temperature[None, :], (HC, HEADS)), np.float32)

    out["ident"] = np.eye(128, dtype=ml_dtypes.bfloat16)
    return out


def prep_x(x):
    """x: (B, 192, 128, 128) f32 -> list of per-core dicts (xa, xb bf16)."""
    B = x.shape[0]
    maps = []
    for b in range(B):
        xf = np.asarray(x[b], np.float32).reshape(C, N).astype(ml_dtypes.bfloat16)
        maps.append({
            "xa": np.ascontiguousarray(xf[:128]),
            "xb": np.ascontiguousarray(xf[128:]),
        })
    return maps


def build(nc):
    """Build the SPMD graph (same graph for every core)."""
    E = {}
    E["xa"] = nc.declare_dram_parameter("xa", [128, N], BF16, isOutput=False)
    E["xb"] = nc.declare_dram_parameter("xb", [64, N], BF16, isOutput=False)
    E["w1Ta"] = nc.declare_dram_parameter("w1Ta", [128, OC], BF16, isOutput=False)
    E["w1Tb"] = nc.declare_dram_parameter("w1Tb", [64, OC], BF16, isOutput=False)
    E["rdiag"] = nc.declare_dram_parameter("rdiag", [128, 27 * 128], BF16, isOutput=False)
    E["rdiagb"] = nc.declare_dram_parameter("rdiagb", [64, 9 * 64], BF16, isOutput=False)
    E["wneg"] = nc.declare_dram_parameter("wneg", [128, 45], F32, isOutput=False)
    E["wtaps"] = nc.declare_dram_parameter("wtaps", [128, 45], F32, isOutput=False)
    E["wpTh"] = nc.declare_dram_parameter("wpTh", [HC, HEADS * C], BF16, isOutput=False)
    E["temps"] = nc.declare_dram_parameter("temps", [HC, HEADS], F32, isOutput=False)
    E["ident"] = nc.declare_dram_parameter("ident", [128, 128], BF16, isOutput=False)
    E["out"] = nc.declare_dram_parameter("out", [C, N], F32, isOutput=True)

    terminals = []

    with tile.TileContext(nc) as tc:
        with contextlib.ExitStack() as ctx:
            _build_body(ctx, tc, nc, E, terminals)

    _split_excess_waits(nc)
    return nc


def _inst_wait_cap(inst):
    # Empirically this walrus build accepts only ONE sem wait per
    # instruction across ISA structs; excess must be split onto NoOps.
    return 1


def _split_excess_waits(nc, maxw_nop=1):
    """Walrus codegen rejects instructions with too many sem waits (1 for
    DMA/CTRL, 2 for TPB compute). Move excess waits onto injected
    same-engine NoOps placed right before the offending instruction (engine
    sequencers execute bb instructions in order, so the waits still
    happen-before)."""
    n_split = 0
    for f in nc.m.functions:
        for bb in f.blocks:
            insts = bb.instructions
            out = []
            changed = False
            for inst in insts:
                si = inst.sync_info
                waits = list(si.on_wait or []) if si else []
                maxw = _inst_wait_cap(inst)
                if len(waits) > maxw:
                    keep = waits[-maxw:]
                    excess = waits[:-maxw]
                    while excess:
                        grp, excess = excess[:maxw_nop], excess[maxw_nop:]
                        n_split += 1
                        nop = mybir.InstEventSemaphore(
                            name=f"wsplit_{n_split}_{inst.name}", ins=[], outs=[])
                        nop.engine = inst.engine
                        nop.debug = inst.debug
                        nop.sync_info = mybir.SyncInfo(on_wait=grp, on_update=[])
                        nc.register_instruction(nop, overwrite=True)
                        out.append(nop)
                    si.on_wait = keep
                    changed = True
                out.append(inst)
            if changed:
                bb.instructions = out


def _build_body(ctx, tc, nc, E, terminals):
    DBG_STRIPS = NSTRIP
    DBG_DW = True
    DBG_GRAM = True
    DBG_PHASEBC = True
    AF = mybir.ActivationFunctionType
    ALU = mybir.AluOpType
    AX = mybir.AxisListType

    singles = ctx.enter_context(tc.tile_pool(name="singles", bufs=1))
    xpool = ctx.enter_context(tc.tile_pool(name="xpool", bufs=2))
    qkv1_pool = ctx.enter_context(tc.tile_pool(name="qkv1", bufs=1))
    qk_pool = ctx.enter_context(tc.tile_pool(name="qk", bufs=2))
    vbar_pool = ctx.enter_context(tc.tile_pool(name="vbar", bufs=1))
    tp_sb_pool = ctx.enter_context(tc.tile_pool(name="tpsb", bufs=3))
    small = ctx.enter_context(tc.tile_pool(name="small", bufs=2))
    scratch = ctx.enter_context(tc.tile_pool(name="scratch", bufs=1))
    outp = ctx.enter_context(tc.tile_pool(name="outp", bufs=2))

    pG = ctx.enter_context(tc.tile_pool(name="pG", bufs=1, space="PSUM"))
    pQkv = ctx.enter_context(tc.tile_pool(name="pQkv", bufs=2, space="PSUM"))
    pDw = ctx.enter_context(tc.tile_pool(name="pDw", bufs=3, space="PSUM"))
    pT = ctx.enter_context(tc.tile_pool(name="pT", bufs=2, space="PSUM"))

    # ---- constants ----
    w1Ta = singles.tile([128, OC], BF16)
    nc.sync.dma_start(out=w1Ta[:], in_=E["w1Ta"][:])
    w1Tb = singles.tile([64, OC], BF16)
    nc.sync.dma_start(out=w1Tb[:], in_=E["w1Tb"][:])
    rdiag = singles.tile([128, 27 * 128], BF16)
    nc.sync.dma_start(out=rdiag[:], in_=E["rdiag"][:])
    rdiagb = singles.tile([64, 9 * 64], BF16)
    nc.sync.dma_start(out=rdiagb[:], in_=E["rdiagb"][:])
    wneg = singles.tile([128, 45], F32)
    nc.sync.dma_start(out=wneg[:], in_=E["wneg"][:])
    wtaps = singles.tile([128, 45], F32)
    nc.sync.dma_start(out=wtaps[:], in_=E["wtaps"][:])
    wpTh = singles.tile([HC, HEADS * C], BF16)
    nc.sync.dma_start(out=wpTh[:], in_=E["wpTh"][:])
    temps = singles.tile([HC, HEADS], F32)
    nc.sync.dma_start(out=temps[:], in_=E["temps"][:])
    ident = singles.tile([128, 128], BF16)
    i_id = nc.sync.dma_start(out=ident[:], in_=E["ident"][:])
    terminals.append(i_id)
    identf = singles.tile([128, 128], F32)
    nc.scalar.copy(out=identf[:], in_=ident[:])

    # persistent slab set
    qkv1_set = [qkv1_pool.tile([sz, PAD], BF16, name=f"qkv1_{i}", tag=f"qkv1_{i}")
                for i, (b, sz) in enumerate(CHUNKS)]
    for i in range(5):
        nc.vector.memset(qkv1_set[i][:, 0:2], 0.0)
        nc.vector.memset(qkv1_set[i][:, PAD - 2:PAD], 0.0)
        nc.vector.memset(qkv1_set[i][:, 2:130], 0.0)  # top halo
    vbar_a = vbar_pool.tile([128, N], BF16)   # v channels 0..127 (global 384..511)
    vbar_b = vbar_pool.tile([64, N], BF16)    # v channels 128..191 (global 512..575)
    nsq = [singles.tile([128, NSTRIP], F32, name=f"nsq{i}", tag=f"nsq{i}") for i in range(3)]

    # G accumulator: (48, heads, 48) f32 in one psum bank, lives all of phase A
    G = pG.tile([HC, HEADS, HC], F32)

    last_pe = last_act = last_dve = None

    # ---------------- phase A: strips ----------------
    for s in range(DBG_STRIPS):
        y0 = s * H_STRIP
        ytop = max(y0 - 1, 0)
        ybot = min(y0 + H_STRIP + 1, HW)  # exclusive
        rows = ybot - ytop
        cols = rows * 128
        wbase = 2 + (ytop - (y0 - 1)) * 128  # slab write base

        xa_t = xpool.tile([128, (H_STRIP + 2) * 128], BF16, tag="xa")
        xb_t = xpool.tile([64, (H_STRIP + 2) * 128], BF16, tag="xb")
        nc.sync.dma_start(out=xa_t[:, :cols], in_=E["xa"][:, ytop * 128:ybot * 128])
        nc.sync.dma_start(out=xb_t[:, :cols], in_=E["xb"][:, ytop * 128:ybot * 128])

        qkv1 = qkv1_set
        if s == NSTRIP - 1:
            for i in range(5):
                nc.vector.memset(qkv1[i][:, 2 + (H_STRIP + 1) * 128:PAD - 2], 0.0)

        # --- qkv 1x1 conv ---
        for i, (cb, csz) in enumerate(CHUNKS):
            nt = 0
            while nt * 512 < cols:
                w = min(512, cols - nt * 512)
                ps = pQkv.tile([csz, 512], F32, tag="pqkv")
                nc.tensor.matmul(ps[:, :w], w1Ta[:, cb:cb + csz],
                                 xa_t[:, nt * 512:nt * 512 + w],
                                 start=True, stop=False)
                nc.tensor.matmul(ps[:, :w], w1Tb[:, cb:cb + csz],
                                 xb_t[:, nt * 512:nt * 512 + w],
                                 start=False, stop=True)
                _w = w
                nc.scalar.copy(
                    out=qkv1[i][:, wbase + nt * 512: wbase + nt * 512 + _w],
                    in_=ps[:, :_w])
                nt += 1

        # --- depthwise 3x3 via diag matmuls; evacuate to qk strips / vbar ---
        qk_sb = [qk_pool.tile([CHUNKS[i][1], OUTC], BF16, name=f"qk{i}", tag=f"qk{i}")
                 for i in range(3)]
        # q/k chunks: diag-matmul dw on TensorE
        NTAP = 9
        for i in ([0, 1, 2] if DBG_DW else []):
            cb, csz = CHUNKS[i]
            for nt in range(OUTC // 512):
                ps = pDw.tile([csz, 512], F32, tag="pdw")
                obase = 2 + 128 + nt * 512
                for t, (dy, dx) in enumerate(TAPS[:NTAP]):
                    if i < 3:
                        lhsT = rdiag[:, (i * 9 + t) * 128:(i * 9 + t + 1) * 128]
                    else:
                        lhsT = rdiagb[:, t * 64:(t + 1) * 64]
                    d = dy * 128 + dx
                    last_pe = nc.tensor.matmul(
                        ps[:], lhsT,
                        qkv1[i][:, obase + d: obase + d + 512],
                        start=(t == 0), stop=(t == NTAP - 1))
                if i < 3:
                    nc.scalar.copy(out=qk_sb[i][:, nt * 512:(nt + 1) * 512], in_=ps[:])
                else:
                    nc.scalar.copy(
                        out=vbar_b[:, y0 * 128 + nt * 512: y0 * 128 + (nt + 1) * 512],
                        in_=ps[:])
        # --- x-edge fixups (q/k chunks; v chunks handled after v-dw) ---
        for i in range(3 if DBG_DW else 0):
            csz = CHUNKS[i][1]
            if i < 3:
                d3 = qk_sb[i].rearrange("p (r x) -> p r x", x=128)
            else:
                vb = vbar_a if i == 3 else vbar_b
                d3 = vb[:, y0 * 128:(y0 + H_STRIP) * 128].rearrange(
                    "p (r x) -> p r x", x=128)
            dst_c0 = d3[:, :, 0:1]
            dst_c127 = d3[:, :, 127:128]
            for dy in (-1, 0, 1):
                t_m1 = TAPS.index((dy, -1))
                t_p1 = TAPS.index((dy, 1))
                a0 = (1 + dy) * 128 + 1
                src0 = qkv1[i][:, a0:a0 + OUTC].rearrange(
                    "p (r x) -> p r x", x=128)[:, :, 0:1]
                last_dve = nc.vector.scalar_tensor_tensor(
                    out=dst_c0, in0=src0,
                    scalar=wneg[:csz, i * 9 + t_m1: i * 9 + t_m1 + 1],
                    in1=dst_c0, op0=ALU.mult, op1=ALU.add)
                # target elements at (dy+2)*128 + 2 + r*128; window the slice
                # 128-aligned-in-length so rearrange stays in bounds.
                a = (dy + 1) * 128 + 4
                src1 = qkv1[i][:, a:a + OUTC] \
                    .rearrange("p (r x) -> p r x", x=128)[:, :, 126:127]
                last_dve = nc.vector.scalar_tensor_tensor(
                    out=dst_c127, in0=src1,
                    scalar=wneg[:csz, i * 9 + t_p1: i * 9 + t_p1 + 1],
                    in1=dst_c127, op0=ALU.mult, op1=ALU.add)

        # --- norms (sum of squares per channel, q/k chunks only) ---
        sq_scr = scratch.tile([128, OUTC], BF16, tag="accB")
        for i in range(3 if DBG_DW else 0):
            csz = CHUNKS[i][1]
            nc.scalar.activation(
                out=sq_scr[:csz], in_=qk_sb[i][:], func=AF.Square,
                accum_out=nsq[i][:csz, s:s + 1])

        # --- transposes + gram, two image rows per iteration ---
        for r2 in range(0, H_STRIP if (DBG_GRAM and DBG_DW) else 0, 2):
            qkT = pT.tile([128, 2, 2 * C], BF16, tag="ptqk")
            for j in (0, 1):
                rsl = slice((r2 + j) * 128, (r2 + j + 1) * 128)
                nc.tensor.transpose(qkT[:, j, 0:128], qk_sb[0][:, rsl], ident[:])
                nc.tensor.transpose(qkT[:, j, 128:256], qk_sb[1][:, rsl], ident[:])
                nc.tensor.transpose(qkT[:, j, 256:384], qk_sb[2][:, rsl], ident[:])
            qkT_sb = tp_sb_pool.tile([128, 2, 2 * C], BF16, tag="qkTsb")
            nc.vector.tensor_copy(out=qkT_sb[:], in_=qkT[:])
            for j in (0, 1):
                first = (s == 0 and r2 == 0 and j == 0)
                last = (s == DBG_STRIPS - 1 and r2 == H_STRIP - 2 and j == 1)
                for h in range(HEADS):
                    nc.tensor.matmul(
                        G[:, h, :], qkT_sb[:, j, h * HC:(h + 1) * HC],
                        qkT_sb[:, j, C + h * HC: C + (h + 1) * HC],
                        start=first, stop=last, skip_group_check=True)

        # v chunks: dw on DVE (tensor_scalar 4x products + tensor_tensor 2x adds)
        SLEN = (H_STRIP + 2) * 128 + 2  # shifted-copy length (even)
        VW = OUTC
        vchain = []  # explicit scheduling order for DVE v-dw ops

        def vop(inst):
            if vchain:
                add_dep_helper(inst.ins, vchain[-1].ins, sync=False,
                               reason="v-dw order")
            vchain.append(inst)
            return inst

        for i in ([3, 4] if DBG_DW else []):
            cb, csz = CHUNKS[i]
            vb = vbar_a if i == 3 else vbar_b
            slab = qkv1[i]
            slab_s = scratch.tile([csz, PAD], BF16, name=f"slabs{i}", tag=f"slabs{i}")
            # slab_s[j] = slab[j+1] (gpsimd: software copy, misalignment ok)
            vop(nc.gpsimd.tensor_copy(out=slab_s[:, 0:SLEN], in_=slab[:, 1:1 + SLEN]))
            obase = 2 + 128
            acc = None
            prods = []
            for t, (dy, dx) in enumerate(TAPS):
                w_ap = wtaps[:csz, i * 9 + t: i * 9 + t + 1]
                if dx == 0:
                    in0 = slab[:, obase + dy * 128: obase + dy * 128 + OUTC]
                elif dx == 1:
                    in0 = slab_s[:, obase + dy * 128: obase + dy * 128 + OUTC]
                else:  # dx == -1
                    in0 = slab_s[:, obase + dy * 128 - 2: obase + dy * 128 - 2 + OUTC]
                p = scratch.tile([csz, OUTC], BF16, name=f"p{i}_{t}", tag=f"p_{t % 3}")
                vop(nc.vector.tensor_scalar_mul(p[:, :VW], in0[:, :VW], w_ap))
                prods.append(p)
                if len(prods) == 3:
                    t0_, t1_, t2_ = prods
                    a = scratch.tile([csz, OUTC], BF16, name=f"a{i}_{t}", tag="a")
                    vop(nc.vector.tensor_add(a[:, :VW], t0_[:, :VW], t1_[:, :VW]))
                    if acc is None:
                        acc = scratch.tile([csz, OUTC], BF16, name=f"acc{i}", tag="acc")
                        vop(nc.vector.tensor_add(acc[:, :VW], a[:, :VW], t2_[:, :VW]))
                    elif t == 8:
                        b_ = scratch.tile([csz, OUTC], BF16, name=f"b{i}_{t}", tag="b")
                        vop(nc.vector.tensor_add(b_[:, :VW], a[:, :VW], t2_[:, :VW]))
                        last_dve = vop(nc.vector.tensor_add(
                            vb[:, y0 * 128: y0 * 128 + VW], acc[:, :VW], b_[:, :VW]))
                    else:
                        b_ = scratch.tile([csz, OUTC], BF16, name=f"b{i}_{t}", tag="b")
                        vop(nc.vector.tensor_add(b_[:, :VW], a[:, :VW], t2_[:, :VW]))
                        acc2 = scratch.tile([csz, OUTC], BF16, name=f"acc2{i}", tag="accB")
                        vop(nc.vector.tensor_add(acc2[:, :VW], acc[:, :VW], b_[:, :VW]))
                        acc = acc2
                    prods = []

        # --- x-edge fixups for v chunks ---
        for i in ([3, 4] if DBG_DW else []):
            csz = CHUNKS[i][1]
            vb = vbar_a if i == 3 else vbar_b
            d3 = vb[:, y0 * 128:(y0 + H_STRIP) * 128].rearrange(
                "p (r x) -> p r x", x=128)
            dst_c0 = d3[:, :, 0:1]
            dst_c127 = d3[:, :, 127:128]
            for dy in (-1, 0, 1):
                t_m1 = TAPS.index((dy, -1))
                t_p1 = TAPS.index((dy, 1))
                a0 = (1 + dy) * 128 + 1
                src0 = qkv1[i][:, a0:a0 + OUTC].rearrange(
                    "p (r x) -> p r x", x=128)[:, :, 0:1]
                last_dve = nc.vector.scalar_tensor_tensor(
                    out=dst_c0, in0=src0,
                    scalar=wneg[:csz, i * 9 + t_m1: i * 9 + t_m1 + 1],
                    in1=dst_c0, op0=ALU.mult, op1=ALU.add)
                a = (dy + 1) * 128 + 4
                src1 = qkv1[i][:, a:a + OUTC] \
                    .rearrange("p (r x) -> p r x", x=128)[:, :, 126:127]
                last_dve = nc.vector.scalar_tensor_tensor(
                    out=dst_c127, in0=src1,
                    scalar=wneg[:csz, i * 9 + t_p1: i * 9 + t_p1 + 1],
                    in1=dst_c127, op0=ALU.mult, op1=ALU.add)

    # ---------------- phase B ----------------
    if not DBG_PHASEBC:
        return
    Gsb = small.tile([HC, HEADS, HC], F32, tag="gsb")
    last_act = nc.scalar.copy(out=Gsb[:], in_=G[:])

    rn = []
    for i in range(3):
        csz = CHUNKS[i][1]
        tot = small.tile([128, 1], F32, tag=f"tot{i}")
        nc.vector.tensor_reduce(out=tot[:csz], in_=nsq[i][:csz], axis=AX.X, op=ALU.add)
        rt = small.tile([128, 1], F32, tag=f"rt{i}")
        nc.scalar.sqrt(out=rt[:csz], in_=tot[:csz])
        rr = small.tile([128, 1], F32, tag=f"rr{i}")
        nc.vector.reciprocal(out=rr[:csz], in_=rt[:csz])
        rn.append(rr)

    def gather_head(dst, global_base):
        """dst (48,1) f32 <- 1/norm for qk-space channels [global_base, +48)."""
        done = 0
        g = global_base
        while done < HC:
            oc, off = g // 128, g % 128
            take = min(HC - done, 128 - off)
            nc.sync.dma_start(out=dst[done:done + take, :],
                              in_=rn[oc][off:off + take, :])
            done += take
            g += take

    mh_sb = []
    for h in range(HEADS):
        rq = small.tile([HC, 1], F32, tag="rq")
        gather_head(rq, h * HC)
        rk = small.tile([HC, 1], F32, tag="rk")
        gather_head(rk, C + h * HC)
        rqt = small.tile([HC, 1], F32, tag="rqt")
        nc.vector.tensor_mul(rqt[:], rq[:], temps[:, h:h + 1])
        # z = (G * rqt[c]) * rk[d]  via transpose sandwich
        z1 = small.tile([HC, HC], F32, tag="z1")
        nc.vector.tensor_scalar_mul(z1[:], Gsb[:, h, :], rqt[:])
        z1T_ps = pT.tile([HC, HC], F32, tag="ptqk")
        nc.tensor.transpose(z1T_ps[:], z1[:], identf[0:HC, 0:HC])
        z1T = small.tile([HC, HC], F32, tag="z1T")
        nc.scalar.copy(out=z1T[:], in_=z1T_ps[:])
        z2 = small.tile([HC, HC], F32, tag="z2")
        nc.vector.tensor_scalar_mul(z2[:], z1T[:], rk[:])
        z2T_ps = pT.tile([HC, HC], F32, tag="ptqk")
        nc.tensor.transpose(z2T_ps[:], z2[:], identf[0:HC, 0:HC])
        z = small.tile([HC, HC], F32, tag="z")
        nc.scalar.copy(out=z[:], in_=z2T_ps[:])
        # softmax along free dim
        m = small.tile([HC, 1], F32, tag="m")
        nc.vector.reduce_max(m[:], z[:], AX.X)
        nm = small.tile([HC, 1], F32, tag="nm")
        nc.vector.tensor_scalar_mul(nm[:], m[:], -1.0)
        e = small.tile([HC, HC], F32, tag="e")
        nc.scalar.activation(out=e[:], in_=z[:], func=AF.Exp, bias=nm[:], scale=1.0)
        ssum = small.tile([HC, 1], F32, tag="ssum")
        nc.vector.reduce_sum(ssum[:], e[:], AX.X)
        rs = small.tile([HC, 1], F32, tag="rs")
        nc.vector.reciprocal(rs[:], ssum[:])
        attn = small.tile([HC, HC], BF16, tag="attn")
        last_dve = nc.vector.tensor_scalar_mul(attn[:], e[:], rs[:])
        # M^T head block (48 d, 192 o) = attn(lhsT).T @ wpTh_h
        mh = pT.tile([HC, C], F32, tag="ptqk")
        nc.tensor.matmul(mh[:], attn[:], wpTh[:, h * C:(h + 1) * C],
                         start=True, stop=True)
        msb = small.tile([HC, C], BF16, tag=f"msb{h}")
        nc.scalar.copy(out=msb[:], in_=mh[:])
        mh_sb.append(msb)

    MTa = singles.tile([128, C], BF16)
    MTb = singles.tile([64, C], BF16)
    nc.sync.dma_start(out=MTa[0:48, :], in_=mh_sb[0][:])
    nc.sync.dma_start(out=MTa[48:96, :], in_=mh_sb[1][:])
    nc.sync.dma_start(out=MTa[96:128, :], in_=mh_sb[2][0:32, :])
    nc.sync.dma_start(out=MTb[0:16, :], in_=mh_sb[2][32:48, :])
    i_m = nc.sync.dma_start(out=MTb[16:64, :], in_=mh_sb[3][:])
    terminals.append(i_m)

    # ---------------- phase C: out = blockdiag-attn-proj @ vbar ----------------
    for nt in range(N // 512):
        sl = slice(nt * 512, (nt + 1) * 512)
        ps0 = pQkv.tile([128, 512], F32, tag="pqkv")
        nc.tensor.matmul(ps0[:], MTa[:, 0:128], vbar_a[:, sl], start=True, stop=False)
        nc.tensor.matmul(ps0[:], MTb[:, 0:128], vbar_b[:, sl], start=False, stop=True)
        ps1 = pDw.tile([64, 512], F32, tag="pdw")
        nc.tensor.matmul(ps1[:], MTa[:, 128:192], vbar_a[:, sl], start=True, stop=False)
        last_pe = nc.tensor.matmul(ps1[:], MTb[:, 128:192], vbar_b[:, sl],
                                   start=False, stop=True)
        o0 = outp.tile([128, 512], F32, tag="o0")
        nc.scalar.copy(out=o0[:], in_=ps0[:])
        o1 = outp.tile([64, 512], F32, tag="o1")
        last_act = nc.scalar.copy(out=o1[:], in_=ps1[:])
        i0 = nc.sync.dma_start(out=E["out"][0:128, sl], in_=o0[:])
        i1 = nc.sync.dma_start(out=E["out"][128:192, sl], in_=o1[:])
        if nt >= N // 512 - 8:
            terminals.append(i0)
            terminals.append(i1)

    terminals.append(last_pe)
    terminals.append(last_act)
    terminals.append(last_dve)


# ----------------------------------------------------------------------------
# Public entry point: full inputs -> full output, 8-way data-parallel over
# batch across NeuronCores 0-7.
# ----------------------------------------------------------------------------

def kernel(x, w_qkv, w_dw, temperature, w_proj):
    from concourse.bass_utils import run_bass_kernel_spmd

    x = np.asarray(x, np.float32)
    B = x.shape[0]
    assert x.shape == (8, C, HW, HW), x.shape

    nc = bass.Bass()
    build(nc)

    wmaps = prep_weights(w_qkv, w_dw, temperature, w_proj)
    xmaps = prep_x(x)
    in_maps = [{**wmaps, **xm} for xm in xmaps]

    res = run_bass_kernel_spmd(nc, in_maps, core_ids=list(range(8)))
    out = np.stack([np.asarray(res.results[b]["out"], np.float32)
                    .reshape(C, HW, HW) for b in range(B)])
    return out

